# revision 3
# baseline (speedup 1.0000x reference)
"""Trainium2 Bass kernel for nn_EquationSampler (RNN equation sampler).

Strategy (pure data parallel over batch n across 8 cores, 8192 samples/core):
  - H state kept hidden-major [128 hidden, 8192 samples] for PE matmuls (fp32).
  - Token-space work done sample-major [128 sample-partitions, 16 tokens, 64]
    so every DVE/ACT op uses all 128 lanes.
  - Constraint masks applied as -1e4 logit biases (masks are 0/1 multiplicative,
    so min(probs, m) == probs * m == softmax(logits - 1e4*(1-m)) renormalized).
  - Sampling: tok = argmax_j q_j * E_j with q = exp(masked logits) and
    E = exp(gumbel) precomputed on host from the exact jax threefry stream.
  - parent_sibling via counter-history matching: i* = max{i<=t: cnt_i == cnt_{t+1}},
    computed with a 15-slot token buffer + power-of-2 weighted select.
  - Layout crossings (token-major <-> sample-major) go through small DRAM
    bounce buffers (DMA on SBUF requires partition dim outermost).
"""
import numpy as np

P = 128
NT = 16
TS = 12          # MAX_STEPS
NCORES = 8
NPC = 8192       # samples per core
NC2 = 64         # free per partition (samples) in sample-major layout
CH = 512         # matmul chunk (columns)
NCH = NPC // CH  # 16 chunks
BIG = -1.0e4

_PROG_CACHE = {}


def _sample_maps():
    """p <-> s mapping. s_local = 512*k + 64*pp + c ; k = 4*lb + q ;
    p = 32*q + 8*lb + pp."""
    p_arr = np.arange(P)
    q = p_arr // 32
    lb = (p_arr // 8) % 4
    pp = p_arr % 8
    k = 4 * lb + q
    sbase_of_p = 512 * k + 64 * pp          # [128] base sample idx for (p, c=0)
    return sbase_of_p


def _build_program():
    from contextlib import ExitStack
    import concourse.bacc as bacc
    import concourse.tile as tile
    from concourse import mybir

    F32 = mybir.dt.float32
    U32 = mybir.dt.uint32
    AL = mybir.AluOpType
    AF = mybir.ActivationFunctionType
    AX = mybir.AxisListType

    nc = bacc.Bacc("TRN2", target_bir_lowering=False, debug=False)

    # ---------------- DRAM parameters ----------------
    Edat = nc.declare_dram_parameter("Edat", [TS, P, NT * NC2], F32, isOutput=False)
    Wh_d = nc.declare_dram_parameter("Whd", [128, 128], F32, isOutput=False)
    Wx4_d = nc.declare_dram_parameter("Wx4d", [128, 128], F32, isOutput=False)
    Wpd_d = nc.declare_dram_parameter("Wpdd", [128, 32], F32, isOutput=False)
    bb_d = nc.declare_dram_parameter("bbd", [128, 2], F32, isOutput=False)
    h0_d = nc.declare_dram_parameter("h0d", [128, 1], F32, isOutput=False)
    cst_d = nc.declare_dram_parameter("cstd", [128, 48], F32, isOutput=False)

    o_seq = nc.declare_dram_parameter("o_seq", [P, TS, NC2], F32, isOutput=True)
    o_lp = nc.declare_dram_parameter("o_lp", [P, TS, NC2], F32, isOutput=True)
    o_ent = nc.declare_dram_parameter("o_ent", [P, TS, NC2], F32, isOutput=True)
    o_act = nc.declare_dram_parameter("o_act", [P, TS, NC2], F32, isOutput=True)
    o_cnt = nc.declare_dram_parameter("o_cnt", [P, NC2], F32, isOutput=True)

    with tile.TileContext(nc) as tc, ExitStack() as ctx:
        # ---------------- pools ----------------
        ones = ctx.enter_context(tc.tile_pool(name="ones", bufs=1))
        st = ctx.enter_context(tc.tile_pool(name="st", bufs=2))        # small state (rebind)
        sm = ctx.enter_context(tc.tile_pool(name="sm", bufs=2))        # small temps
        hpool = ctx.enter_context(tc.tile_pool(name="hpool", bufs=2))
        xop = ctx.enter_context(tc.tile_pool(name="xop", bufs=2))
        tkp = ctx.enter_context(tc.tile_pool(name="tkp", bufs=2))      # token tiles
        tk1 = ctx.enter_context(tc.tile_pool(name="tk1", bufs=1))      # token tiles (single)
        epool = ctx.enter_context(tc.tile_pool(name="epool", bufs=3))
        lstp = ctx.enter_context(tc.tile_pool(name="lstp", bufs=1))
        eqpp = ctx.enter_context(tc.tile_pool(name="eqpp", bufs=1))
        msp = ctx.enter_context(tc.tile_pool(name="msp", bufs=1))      # m / sel
        dram = ctx.enter_context(tc.tile_pool(name="dram", bufs=2, space="DRAM"))
        hps = ctx.enter_context(tc.tile_pool(name="hps", bufs=4, space="PSUM"))
        lps = ctx.enter_context(tc.tile_pool(name="lps", bufs=2, space="PSUM"))

        # ---------------- constants & weights ----------------
        cst = ones.tile([P, 48], F32)
        nc.sync.dma_start(out=cst[:], in_=cst_d[:])
        Wh_s = ones.tile([128, 128], F32)
        nc.sync.dma_start(out=Wh_s[:], in_=Wh_d[:])
        Wx4_s = ones.tile([128, 128], F32)
        nc.sync.dma_start(out=Wx4_s[:], in_=Wx4_d[:])
        Wpd_s = ones.tile([128, 32], F32)
        nc.sync.dma_start(out=Wpd_s[:], in_=Wpd_d[:])
        bb_s = ones.tile([128, 2], F32)
        nc.sync.dma_start(out=bb_s[:], in_=bb_d[:])
        h0_s = ones.tile([128, 1], F32)
        nc.sync.dma_start(out=h0_s[:], in_=h0_d[:])

        REVJ = cst[:, 0:16]     # 15 - j
        JVAL = cst[:, 16:32]    # j
        POW2 = cst[:, 32:46]    # 2^k, k=0..13
        NEG1 = cst[:, 46:47]

        # ---------------- persistent state ----------------
        seqsl = ones.tile([P, 15, NC2], F32)
        nc.vector.memset(seqsl[:], -1.0)
        Ph = ones.tile([P, TS, NC2], F32)
        wbuf = ones.tile([P, 14, NC2], F32)
        nc.vector.memset(wbuf[:, 0:1, :], 1.0)   # phantom weight
        tok0p1 = ones.tile([P, NC2], F32)

        acc_seq = ones.tile([P, TS, NC2], F32)
        acc_lp = ones.tile([P, TS, NC2], F32)
        acc_act = ones.tile([P, TS, NC2], F32)
        acc_S = ones.tile([P, TS, NC2], F32)
        acc_T = ones.tile([P, TS, NC2], F32)

        cnt = st.tile([P, NC2], F32, tag="cnt")
        nc.vector.memset(cnt[:], 1.0)
        cl = st.tile([P, NC2], F32, tag="cl")
        nc.vector.memset(cl[:], 1.0)
        actv = st.tile([P, NC2], F32, tag="actv")
        nc.vector.memset(actv[:], 1.0)
        hasv = st.tile([P, NC2], F32, tag="hasv")
        nc.vector.memset(hasv[:], 0.0)
        bB = None
        bC = None

        # initial H (all columns identical = init_hidden)
        H = hpool.tile([128, NPC], F32, tag="H")
        nc.scalar.activation(H[:], h0_s[:].broadcast_to([128, NPC]), AF.Copy)
        XO = None

        for t in range(TS):
            # ---------- E prefetch ----------
            Et = epool.tile([P, NT * NC2], F32, tag="Et")
            nc.sync.dma_start(out=Et[:], in_=Edat[t])
            Etv = Et[:].rearrange("p (j c) -> p j c", j=NT)

            # ---------- Ph slot (counter entering step t) ----------
            nc.scalar.copy(Ph[:, t, :], cnt[:])

            # ---------- RNN update ----------
            Hn = hpool.tile([128, NPC], F32, tag="H")
            for kb in range(4):
                pss = []
                for g in range(4):
                    k = 4 * kb + g
                    ps = hps.tile([128, CH], F32, tag="hps")
                    nc.tensor.matmul(ps[:], Wh_s[:], H[:, CH * k:CH * (k + 1)],
                                     start=True, stop=(t == 0))
                    pss.append(ps)
                if t > 0:
                    for g in range(4):
                        nc.tensor.matmul(
                            pss[g][:], Wx4_s[32 * g:32 * g + 32, :],
                            XO[32 * g:32 * g + 32, CH * kb:CH * (kb + 1)],
                            start=False, stop=True, tile_position=(32 * g, 0))
                for g in range(4):
                    k = 4 * kb + g
                    nc.scalar.activation(Hn[:, CH * k:CH * (k + 1)], pss[g][:],
                                         AF.Tanh, bias=bb_s[:, 1:2] if t else bb_s[:, 0:1],
                                         scale=1.0)
            H = Hn

            # ---------- logits ----------
            Lst = lstp.tile([128, 4 * CH], F32, tag="Lst")
            for lb in range(4):
                psl = lps.tile([128, CH], F32, tag="lps")
                for qd in range(4):
                    k = 4 * lb + qd
                    nc.tensor.matmul(psl[32 * qd:32 * qd + 32, :], Wpd_s[:],
                                     H[:, CH * k:CH * (k + 1)],
                                     start=True, stop=True, tile_position=(0, 32 * qd))
                nc.scalar.copy(Lst[:, CH * lb:CH * (lb + 1)], psl[:])
            Ld = dram.tile([128, 4 * CH], F32, tag="Ld")
            nc.sync.dma_start(out=Ld[:], in_=Lst[:])
            Ttok = tkp.tile([P, NT, NC2], F32, tag="Ttok")
            # dst partitions [32q, 32q+32) <- Ldram rows 32q+j, cols 512*lb+64*pp+c
            for q in range(4):
                src = Ld[32 * q:32 * q + 16, :].rearrange(
                    "j (lb pp c) -> (lb pp) j c", lb=4, pp=8)
                nc.sync.dma_start(out=Ttok[32 * q:32 * q + 32, :, :], in_=src)

            # ---------- constraint biases ----------
            if t == 0:
                nc.vector.tensor_scalar(out=Ttok[:, 8:16, :], in0=Ttok[:, 8:16, :],
                                        scalar1=BIG, scalar2=None, op0=AL.add)
            else:
                if t >= 5:
                    nc.vector.tensor_tensor(
                        out=Ttok[:, 0:8, :], in0=Ttok[:, 0:8, :],
                        in1=bB[:, None, :].broadcast_to([P, 8, NC2]), op=AL.add)
                nc.vector.tensor_tensor(
                    out=Ttok[:, 8:9, :], in0=Ttok[:, 8:9, :],
                    in1=bC[:, None, :].broadcast_to([P, 1, NC2]), op=AL.add)

            # ---------- q, score, argmax ----------
            qt = tkp.tile([P, NT, NC2], F32, tag="qt")
            nc.scalar.activation(qt[:], Ttok[:], AF.Exp)
            score = tk1.tile([P, NT, NC2], F32, tag="score")
            nc.vector.tensor_tensor(out=score[:], in0=qt[:], in1=Etv, op=AL.mult)
            vmax = sm.tile([P, NC2], F32, tag="vmax")
            nc.vector.tensor_reduce(out=vmax[:], in_=score[:].transpose([0, 2, 1]),
                                    axis=AX.X, op=AL.max)
            eqm = tk1.tile([P, NT, NC2], F32, tag="eqm")
            nc.vector.tensor_tensor(out=eqm[:], in0=score[:],
                                    in1=vmax[:, None, :].broadcast_to([P, NT, NC2]),
                                    op=AL.is_equal)
            tokm = tk1.tile([P, NT, NC2], F32, tag="tokm")
            nc.vector.tensor_tensor(out=tokm[:], in0=eqm[:],
                                    in1=REVJ[:, :, None].broadcast_to([P, NT, NC2]),
                                    op=AL.mult)
            tokf = sm.tile([P, NC2], F32, tag="tokf")
            nc.vector.tensor_reduce(out=tokf[:], in_=tokm[:].transpose([0, 2, 1]),
                                    axis=AX.X, op=AL.max)
            tok = sm.tile([P, NC2], F32, tag="tok")
            nc.vector.tensor_scalar(out=tok[:], in0=tokf[:], scalar1=-1.0,
                                    scalar2=15.0, op0=AL.mult, op1=AL.add)

            # ---------- S, T, ltok ----------
            nc.vector.tensor_reduce(out=acc_S[:, t, :], in_=qt[:].transpose([0, 2, 1]),
                                    axis=AX.X, op=AL.add)
            tmq = tk1.tile([P, NT, NC2], F32, tag="tmq")
            nc.vector.tensor_tensor(out=tmq[:], in0=qt[:], in1=Ttok[:], op=AL.mult)
            nc.vector.tensor_reduce(out=acc_T[:, t, :], in_=tmq[:].transpose([0, 2, 1]),
                                    axis=AX.X, op=AL.add)
            tml = tk1.tile([P, NT, NC2], F32, tag="tml")
            nc.vector.tensor_tensor(out=tml[:], in0=eqm[:], in1=Ttok[:], op=AL.mult)
            nc.vector.tensor_reduce(out=acc_lp[:, t, :], in_=tml[:].transpose([0, 2, 1]),
                                    axis=AX.X, op=AL.add)

            # ---------- counters ----------
            lt4 = sm.tile([P, NC2], F32, tag="lt4")
            nc.vector.tensor_scalar(out=lt4[:], in0=tok[:], scalar1=4.0, scalar2=None,
                                    op0=AL.is_lt)
            lt8 = sm.tile([P, NC2], F32, tag="lt8")
            nc.vector.tensor_scalar(out=lt8[:], in0=tok[:], scalar1=8.0, scalar2=None,
                                    op0=AL.is_lt)
            arity = sm.tile([P, NC2], F32, tag="arity")
            nc.vector.tensor_tensor(out=arity[:], in0=lt4[:], in1=lt8[:], op=AL.add)
            cnt2 = st.tile([P, NC2], F32, tag="cnt")
            nc.vector.scalar_tensor_tensor(out=cnt2[:], in0=cnt[:], scalar=-1.0,
                                           in1=arity[:], op0=AL.add, op1=AL.add)
            cl2 = st.tile([P, NC2], F32, tag="cl")
            nc.vector.tensor_tensor(out=cl2[:], in0=cl[:], in1=arity[:], op=AL.add)
            gt0 = sm.tile([P, NC2], F32, tag="gt0")
            nc.vector.tensor_scalar(out=gt0[:], in0=cnt2[:], scalar1=0.0, scalar2=None,
                                    op0=AL.is_gt)
            actv2 = st.tile([P, NC2], F32, tag="actv")
            nc.vector.tensor_tensor(out=actv2[:], in0=actv[:], in1=gt0[:], op=AL.mult)
            ge9 = sm.tile([P, NC2], F32, tag="ge9")
            nc.vector.tensor_scalar(out=ge9[:], in0=tok[:], scalar1=9.0, scalar2=None,
                                    op0=AL.is_ge)
            hasv2 = st.tile([P, NC2], F32, tag="hasv")
            nc.vector.tensor_tensor(out=hasv2[:], in0=hasv[:], in1=ge9[:], op=AL.max)

            nc.scalar.copy(acc_seq[:, t, :], tok[:])
            nc.scalar.copy(acc_act[:, t, :], actv2[:])
            nc.scalar.copy(seqsl[:, 2 + t, :], tok[:])
            if t == 0:
                nc.vector.tensor_scalar(out=tok0p1[:], in0=tok[:], scalar1=1.0,
                                        scalar2=None, op0=AL.add)

            # ---------- parent / sibling ----------
            mten = msp.tile([P, TS, NC2], F32, tag="mten")
            nc.vector.tensor_tensor(
                out=mten[:, 0:t + 1, :], in0=Ph[:, 0:t + 1, :],
                in1=cnt2[:, None, :].broadcast_to([P, t + 1, NC2]), op=AL.is_equal)
            nc.vector.tensor_tensor(
                out=wbuf[:, 1:t + 2, :], in0=mten[:, 0:t + 1, :],
                in1=POW2[:, 1:t + 2, None].broadcast_to([P, t + 1, NC2]), op=AL.mult)
            mx = sm.tile([P, NC2], F32, tag="mx")
            nc.vector.tensor_reduce(out=mx[:], in_=wbuf[:, 0:t + 2, :].transpose([0, 2, 1]),
                                    axis=AX.X, op=AL.max)
            sel = msp.tile([P, 14, NC2], F32, tag="sel")
            nc.vector.tensor_tensor(
                out=sel[:, 0:t + 2, :], in0=wbuf[:, 0:t + 2, :],
                in1=mx[:, None, :].broadcast_to([P, t + 2, NC2]), op=AL.is_equal)
            pmul = msp.tile([P, 14, NC2], F32, tag="pmul")
            nc.vector.tensor_tensor(out=pmul[:, 0:t + 2, :], in0=sel[:, 0:t + 2, :],
                                    in1=seqsl[:, 1:t + 3, :], op=AL.mult)
            PS2 = sm.tile([P, 2, NC2], F32, tag="PS2")
            nc.vector.tensor_reduce(out=PS2[:, 0, :],
                                    in_=pmul[:, 0:t + 2, :].transpose([0, 2, 1]),
                                    axis=AX.X, op=AL.add)
            smul = msp.tile([P, 14, NC2], F32, tag="smul")
            nc.vector.tensor_tensor(out=smul[:, 0:t + 2, :], in0=sel[:, 0:t + 2, :],
                                    in1=seqsl[:, 2:t + 4, :], op=AL.mult)
            sibr = sm.tile([P, NC2], F32, tag="sibr")
            nc.vector.tensor_reduce(out=sibr[:],
                                    in_=smul[:, 0:t + 2, :].transpose([0, 2, 1]),
                                    axis=AX.X, op=AL.add)
            corr = sm.tile([P, NC2], F32, tag="corr")
            nc.vector.tensor_tensor(out=corr[:], in0=sel[:, 0, :], in1=tok0p1[:],
                                    op=AL.mult)
            nc.vector.tensor_tensor(out=PS2[:, 1, :], in0=sibr[:], in1=corr[:],
                                    op=AL.subtract)
            isop = sm.tile([P, NC2], U32, tag="isop")
            nc.vector.tensor_scalar(out=isop[:], in0=tok[:], scalar1=8.0, scalar2=None,
                                    op0=AL.is_lt)
            nc.vector.copy_predicated(PS2[:, 0, :], isop[:], tok[:])
            nc.vector.copy_predicated(PS2[:, 1, :], isop[:],
                                      NEG1[:].broadcast_to([P, NC2]))

            # ---------- next-step biases ----------
            bB = sm.tile([P, NC2], F32, tag="bB")
            nc.vector.tensor_scalar(out=bB[:], in0=cl2[:], scalar1=10.0, scalar2=BIG,
                                    op0=AL.is_gt, op1=AL.mult)
            ceq1 = sm.tile([P, NC2], F32, tag="ceq1")
            nc.vector.tensor_scalar(out=ceq1[:], in0=cnt2[:], scalar1=1.0, scalar2=None,
                                    op0=AL.is_equal)
            novar = sm.tile([P, NC2], F32, tag="novar")
            nc.vector.tensor_scalar(out=novar[:], in0=hasv2[:], scalar1=0.5, scalar2=None,
                                    op0=AL.is_lt)
            bC = sm.tile([P, NC2], F32, tag="bC")
            nc.vector.scalar_tensor_tensor(out=bC[:], in0=ceq1[:], scalar=BIG,
                                           in1=novar[:], op0=AL.mult, op1=AL.mult)

            # ---------- one-hot for next x + transport ----------
            if t < TS - 1:
                eqp = eqpp.tile([P, 2, NT, NC2], F32, tag="eqp")
                nc.vector.tensor_tensor(
                    out=eqp[:], in0=PS2[:, :, None, :].broadcast_to([P, 2, NT, NC2]),
                    in1=JVAL[:, None, :, None].broadcast_to([P, 2, NT, NC2]),
                    op=AL.is_equal)
                Xd = dram.tile([P, 2 * NT * NC2], F32, tag="Xd")
                nc.sync.dma_start(out=Xd[:], in_=eqp[:])
                XOn = xop.tile([128, 4 * CH], F32, tag="XO")
                for q in range(4):
                    src = Xd[32 * q:32 * q + 32, :].rearrange(
                        "lp (mj c) -> mj lp c", mj=32)
                    nc.sync.dma_start(out=XOn[32 * q:32 * q + 32, :], in_=src)
                XO = XOn

            cnt, cl, actv, hasv = cnt2, cl2, actv2, hasv2

        # ---------------- finalize lp / ent ----------------
        lnS = ones.tile([P, TS, NC2], F32)
        nc.scalar.activation(lnS[:], acc_S[:], AF.Ln)
        nc.vector.tensor_tensor(out=acc_lp[:], in0=acc_lp[:], in1=lnS[:], op=AL.subtract)
        rS = ones.tile([P, TS, NC2], F32)
        nc.vector.reciprocal(rS[:], acc_S[:])
        ToS = ones.tile([P, TS, NC2], F32)
        nc.vector.tensor_tensor(out=ToS[:], in0=acc_T[:], in1=rS[:], op=AL.mult)
        acc_ent = ones.tile([P, TS, NC2], F32)
        nc.vector.tensor_tensor(out=acc_ent[:], in0=lnS[:], in1=ToS[:], op=AL.subtract)

        nc.sync.dma_start(out=o_seq[:], in_=acc_seq[:])
        nc.sync.dma_start(out=o_lp[:], in_=acc_lp[:])
        nc.sync.dma_start(out=o_ent[:], in_=acc_ent[:])
        nc.sync.dma_start(out=o_act[:], in_=acc_act[:])
        nc.sync.dma_start(out=o_cnt[:], in_=cnt[:])

    nc.compile()
    return nc


def _get_program():
    if "nc" not in _PROG_CACHE:
        _PROG_CACHE["nc"] = _build_program()
    return _PROG_CACHE["nc"]


def kernel(n, input_tensor0, init_hidden0, Wx, Wh, b, Wp, bp, _profile=False):
    import jax
    import jax.numpy as jnp
    from concourse.bass_utils import run_bass_kernel_spmd

    n = int(n)
    f32 = np.float32
    Wx = np.asarray(Wx, f32)
    Wh = np.asarray(Wh, f32)
    b = np.asarray(b, f32)
    Wp = np.asarray(Wp, f32)
    bp = np.asarray(bp, f32)
    x0 = np.asarray(input_tensor0, f32)
    h0 = np.asarray(init_hidden0, f32)
    if np.any(bp != 0):
        # not exercised by this problem's inputs; fold would need an extra op
        raise NotImplementedError("nonzero bp not supported")

    ntot = NCORES * NPC
    # ---------------- host gumbel precompute (exact jax stream) ----------------
    cpu = jax.local_devices(backend="cpu")[0]
    with jax.default_device(cpu):
        keys = jax.random.split(jax.random.key(42), TS)
        G = np.stack([np.asarray(jax.random.gumbel(k, (n, NT), jnp.float32))
                      for k in keys])            # [TS, n, 16]
    E = np.exp(G.astype(f32)).astype(f32)
    if n < ntot:
        E = np.concatenate([E, np.ones((TS, ntot - n, NT), f32)], axis=1)

    sbase_of_p = _sample_maps()                   # [128]
    # sample index grid per core: s_local[p, c] = sbase_of_p[p] + c
    s_grid = (sbase_of_p[:, None] + np.arange(NC2)[None, :])   # [128, 64]

    b0 = (b + (x0 @ Wx)[0]).astype(f32)
    bb = np.stack([b0, b], axis=1)               # [128, 2]
    Wx4 = np.tile(Wx, (4, 1)).astype(f32)        # [128, 128]
    Wpd = np.concatenate([Wp, Wp], axis=1).astype(f32)  # [128, 32]
    h0c = h0.reshape(128, 1)

    cstv = np.zeros((48,), f32)
    cstv[0:16] = 15.0 - np.arange(16)
    cstv[16:32] = np.arange(16)
    cstv[32:46] = 2.0 ** np.arange(14)
    cstv[46] = -1.0
    cst = np.tile(cstv[None, :], (P, 1)).astype(f32)

    in_maps = []
    for d in range(NCORES):
        idx = d * NPC + s_grid                    # [128, 64] global sample ids
        Ecore = E[:, idx, :]                      # [TS, 128, 64, 16]
        Ecore = np.ascontiguousarray(Ecore.transpose(0, 1, 3, 2)  # [TS,128,16,64]
                                     ).reshape(TS, P, NT * NC2)
        in_maps.append({
            "Edat": Ecore, "Whd": Wh, "Wx4d": Wx4, "Wpdd": Wpd,
            "bbd": bb, "h0d": h0c, "cstd": cst,
        })

    nc = _get_program()
    kw = {}
    if _profile:
        kw = dict(trace=True)
    res = run_bass_kernel_spmd(nc, in_maps, list(range(NCORES)), **kw)
    results = res.results

    # ---------------- unshard ----------------
    flat_idx = s_grid.reshape(-1)                 # local sample id for (p*64+c)
    inv = np.empty(NPC, np.int64)
    inv[flat_idx] = np.arange(NPC)

    def gather(name, steps=True):
        outs = []
        for d in range(NCORES):
            a = results[d][name]
            if steps:
                a = a.transpose(0, 2, 1).reshape(NPC, TS)  # [p,c,t] -> [s, t]
            else:
                a = a.reshape(NPC)
            outs.append(a[inv])
        return np.concatenate(outs, axis=0)[:n]

    seq = np.rint(gather("o_seq")).astype(np.int32)
    lp = gather("o_lp").astype(f32)
    ent = gather("o_ent").astype(f32)
    acts = np.rint(gather("o_act")).astype(np.int32)
    cntf = np.rint(gather("o_cnt", steps=False)).astype(np.int32)

    mask = np.concatenate([np.ones((n, 1), np.int32), acts], axis=1).astype(bool)
    lengths = mask.astype(np.int32).sum(axis=1)
    if _profile:
        return (seq, ent, lp, cntf, lengths, mask), res
    return seq, ent, lp, cntf, lengths, mask
